# revision 17
# baseline (speedup 1.0000x reference)
# Trainium2 Bass kernel for nn_Attention_57509612094021 (XCA-style channel attention).
#
# Sharding: pure data-parallel over batch (8 images -> 8 NeuronCores), no collectives.
# Per-core pipeline (one [192,128,128] image):
#   - stream 8-row blocks: 1x1 convs (fp32r matmuls on PE) -> padded SBUF tiles,
#     depthwise 3x3 split between DVE scalar_tensor_tensor chains (bf16 2x mode,
#     alignment fixed via a 1-shifted copy) and PE diag-matmul accumulation,
#   - per-block DMA-xbar transposes of q|k -> stacked per-head [96,96] Gram
#     accumulated in PSUM across the whole image (diagonal = L2 norms),
#   - softmax (exp on ACT) -> attn @ v -> 1x1 proj, streamed back out.
import os
import sys
import time

sys.path.insert(0, "/opt/trn_rl_repo")
os.environ.setdefault("JAX_PLATFORMS", "axon")

import numpy as np
import ml_dtypes

import concourse.bass as bass
import concourse.tile as tile
from concourse import bacc, mybir
from concourse.bass_utils import run_bass_kernel_spmd

F32 = mybir.dt.float32
F32R = mybir.dt.float32r
BF16 = mybir.dt.bfloat16
AF = mybir.ActivationFunctionType
OP = mybir.AluOpType
bf16 = ml_dtypes.bfloat16

C = 192
O = 384  # 2C
H = W = 128
HW = H * W
HEADS = 4
CPH = 48
R = 8            # rows per block
NB = H // R      # 16 blocks
PXB = R * W      # 1024 pixels per block
TAPS = [(ky, kx) for ky in range(3) for kx in range(3)]

# depthwise engine split: v-slab0 (128ch) all-PE; v-slab1 (64ch) PE for kx!=1,
# DVE (aligned, no B needed) for kx==1; qk slabs all-DVE.
V1_PE_TAPS = [t for t, (ky, kx) in enumerate(TAPS) if kx != 1]
V1_DVE_TAPS = [t for t, (ky, kx) in enumerate(TAPS) if kx == 1]


def build_nc():
    nc = bacc.Bacc("TRN2", target_bir_lowering=False, debug=False, num_devices=8)

    d_x = nc.dram_tensor("x", [C, HW], F32R, kind="ExternalInput").ap()
    d_wqkT = nc.dram_tensor("wqkT", [C, O], F32R, kind="ExternalInput").ap()
    d_wvT = nc.dram_tensor("wvT", [C, 256], F32R, kind="ExternalInput").ap()
    d_wprojT = nc.dram_tensor("wprojT", [256, C], BF16, kind="ExternalInput").ap()
    d_dwqk = nc.dram_tensor("dwqk", [O, 9], F32, kind="ExternalInput").ap()
    d_dwv = nc.dram_tensor("dwv", [256, 9], F32, kind="ExternalInput").ap()
    d_diagv0 = nc.dram_tensor("diagv0", [128, 9 * 128], BF16, kind="ExternalInput").ap()
    d_diagv1 = nc.dram_tensor("diagv1", [128, 9 * 128], BF16, kind="ExternalInput").ap()
    d_gmask = nc.dram_tensor("gmask", [96, 4 * 96], F32, kind="ExternalInput").ap()
    d_tmmask = nc.dram_tensor("tmmask", [96, 4], F32, kind="ExternalInput").ap()
    d_idbf = nc.dram_tensor("idbf", [128, 128], BF16, kind="ExternalInput").ap()
    d_idf32 = nc.dram_tensor("idf32", [128, 128], F32, kind="ExternalInput").ap()
    d_hsel = nc.dram_tensor("hsel", [4, 4 * 48], F32, kind="ExternalInput").ap()
    d_y = nc.dram_tensor("y", [C, HW], F32, kind="ExternalOutput").ap()

    with tile.TileContext(nc) as tc:
        # ---------------- pools ----------------
        consts = tc.alloc_tile_pool(name="consts", bufs=1)
        persist = tc.alloc_tile_pool(name="persist", bufs=1)
        xpool = tc.alloc_tile_pool(name="xpool", bufs=2)
        blkpool = tc.alloc_tile_pool(name="blkpool", bufs=2)
        scpool = tc.alloc_tile_pool(name="scpool", bufs=1)
        pb_pool = tc.alloc_tile_pool(name="pbpool", bufs=2)
        ps_gram = tc.alloc_tile_pool(name="ps_gram", bufs=1, space="PSUM")
        ps_conv = tc.alloc_tile_pool(name="ps_conv", bufs=2, space="PSUM")
        ps_dw = tc.alloc_tile_pool(name="ps_dw", bufs=2, space="PSUM")

        # ---------------- constants / weights ----------------
        wqkT_a = consts.tile([128, O], F32R, tag="wqkT_a")
        wqkT_b = consts.tile([64, O], F32R, tag="wqkT_b")
        wvT_a = consts.tile([128, 256], F32R, tag="wvT_a")
        wvT_b = consts.tile([64, 256], F32R, tag="wvT_b")
        wppad_a = consts.tile([128, C], BF16, tag="wppad_a")
        wppad_b = consts.tile([128, C], BF16, tag="wppad_b")
        dwqk = [consts.tile([128, 9], F32, tag=f"dwqk{s}", name=f"dwqk{s}") for s in range(3)]
        dwv0 = consts.tile([128, 9], F32, tag="dwv0")
        dwv1 = consts.tile([128, 9], F32, tag="dwv1")
        diagv0 = consts.tile([128, 9, 128], BF16, tag="diagv0")
        diagv1 = consts.tile([128, 9, 128], BF16, tag="diagv1")
        gmask = consts.tile([96, 4 * 96], F32, tag="gmask")
        tmmask = consts.tile([96, 4], F32, tag="tmmask")
        idbf = consts.tile([128, 128], BF16, tag="idbf")
        idf32 = consts.tile([128, 128], F32, tag="idf32")
        hsel = consts.tile([4, 4 * 48], F32, tag="hsel")

        nc.sync.dma_start(wqkT_a[:], d_wqkT[0:128, :])
        nc.sync.dma_start(wqkT_b[:], d_wqkT[128:192, :])
        nc.sync.dma_start(wvT_a[:], d_wvT[0:128, :])
        nc.sync.dma_start(wvT_b[:], d_wvT[128:192, :])
        nc.sync.dma_start(wppad_a[:], d_wprojT[0:128, :])
        nc.sync.dma_start(wppad_b[:], d_wprojT[128:256, :])
        for s in range(3):
            nc.sync.dma_start(dwqk[s][:], d_dwqk[128 * s : 128 * (s + 1), :])
        nc.sync.dma_start(dwv0[:], d_dwv[0:128, :])
        nc.sync.dma_start(dwv1[:], d_dwv[128:256, :])
        nc.sync.dma_start(diagv0[:], d_diagv0[:].rearrange("p (t c) -> p t c", t=9))
        nc.sync.dma_start(diagv1[:], d_diagv1[:].rearrange("p (t c) -> p t c", t=9))
        nc.sync.dma_start(gmask[:], d_gmask[:])
        nc.sync.dma_start(tmmask[:], d_tmmask[:])
        nc.sync.dma_start(idbf[:], d_idbf[:])
        nc.sync.dma_start(idf32[:], d_idf32[:])
        nc.sync.dma_start(hsel[:], d_hsel[:])

        # preload ACT table sets used later (exp loads at use; sqrt preloaded here)
        actwarm = consts.tile([1, 8], F32, tag="actwarm")
        nc.vector.memset(actwarm[:], 1.0)
        nc.scalar.activation(actwarm[:], actwarm[:], AF.Sqrt)

        # ---------------- persistent tensors ----------------
        v_buf = [
            persist.tile([128, HW], BF16, tag="v_buf0", name="v_buf0"),
            persist.tile([128, HW], BF16, tag="v_buf1", name="v_buf1"),
        ]
        gram = ps_gram.tile([96, 4 * 96], F32, tag="gram")

        # padded pre-activation buffers, manual parity double-buffer
        # qk: 3 slabs; v: 2 slabs. A = base, B = 1-shifted copy (qk only).
        PADW = 132
        NPR = R + 2
        padqkA = [[persist.tile([128, NPR, PADW], BF16, tag=f"pqA{s}{p}", name=f"pqA{s}{p}") for p in range(2)] for s in range(3)]
        padqkB = [[persist.tile([128, NPR, PADW - 1], BF16, tag=f"pqB{s}{p}", name=f"pqB{s}{p}") for p in range(2)] for s in range(3)]
        padvA = [
            [persist.tile([128, NPR, PADW], BF16, tag=f"pvA0{p}", name=f"pvA0{p}") for p in range(2)],
            [persist.tile([128, NPR, PADW], BF16, tag=f"pvA1{p}", name=f"pvA1{p}") for p in range(2)],
        ]
        # zero the side columns once (cols 0,1,130,131 never written later)
        for s in range(3):
            for p in range(2):
                nc.vector.memset(padqkA[s][p][:, :, 0:2], 0.0)
                nc.vector.memset(padqkA[s][p][:, :, 130:132], 0.0)
        for s in range(2):
            for p in range(2):
                nc.vector.memset(padvA[s][p][:, :, 0:2], 0.0)
                nc.vector.memset(padvA[s][p][:, :, 130:132], 0.0)
        # zero halo rows used by first block (parity 0)
        for s in range(3):
            nc.vector.memset(padqkA[s][0][:, 0:1, :], 0.0)
        for s in range(2):
            nc.vector.memset(padvA[s][0][:, 0:1, :], 0.0)

        dwsc = [scpool.tile([128, PXB], BF16, tag=f"dwsc{i}", name=f"dwsc{i}") for i in range(2)]

        # ---------------- phase A: blocks ----------------
        for b in range(NB):
            par = b % 2
            r0 = b * R
            lo = max(r0 - 1, 0)
            hi = min(r0 + R, H - 1)
            nr = hi - lo + 1
            row_off = lo - (r0 - 1)  # 1 for b==0 else 0
            npx = nr * W

            if b == NB - 1:
                # zero the bottom halo row (stale from block b-2)
                for s in range(3):
                    nc.vector.memset(padqkA[s][par][:, R + 1 : R + 2, :], 0.0)
                for s in range(2):
                    nc.vector.memset(padvA[s][par][:, R + 1 : R + 2, :], 0.0)

            x_a = xpool.tile([128, npx], F32R, tag="x_a")
            x_b = xpool.tile([64, npx], F32R, tag="x_b")
            nc.sync.dma_start(x_a[:], d_x[0:128, lo * W : (hi + 1) * W])
            nc.sync.dma_start(x_b[:], d_x[128:192, lo * W : (hi + 1) * W])

            # conv chunk row split
            if nr == 10:
                chunks = [(0, 4), (4, 3), (7, 3)]
            else:
                chunks = [(0, 4), (4, 3), (7, 2)]

            def conv_to_pad(wa, wb, mlo, mhi, dst, n_mpart):
                # output channels [mlo:mhi) -> dst pad tile ([n_mpart, NPR, PADW])
                for (cr0, crn) in chunks:
                    ps = ps_conv.tile([128, 4, W], F32, tag="conv")
                    pss = ps[:n_mpart, :crn, :]
                    rhs_a = x_a[:, cr0 * W : (cr0 + crn) * W]
                    rhs_b = x_b[:, cr0 * W : (cr0 + crn) * W]
                    nc.tensor.matmul(pss, wa[:, mlo:mhi], rhs_a, start=True, stop=False)
                    nc.tensor.matmul(pss, wb[:, mlo:mhi], rhs_b, start=False, stop=True)
                    nc.scalar.copy(
                        dst[:n_mpart, row_off + cr0 : row_off + cr0 + crn, 2 : 2 + W], pss
                    )

            for s in range(3):
                conv_to_pad(wqkT_a, wqkT_b, 128 * s, 128 * (s + 1), padqkA[s][par], 128)
            conv_to_pad(wvT_a, wvT_b, 0, 128, padvA[0][par], 128)
            conv_to_pad(wvT_a, wvT_b, 128, 256, padvA[1][par], 128)

            # B copies (1-shifted) for qk slabs
            for s in range(3):
                nc.vector.tensor_copy(
                    padqkB[s][par][:, :, 0:130], padqkA[s][par][:, :, 1:131]
                )

            def win(padA, padB, npart, t, r_lo, rn):
                # input rows for out pad rows [1+r_lo, 1+r_lo+rn) are
                # [ky+r_lo, ky+r_lo+rn); input col base is 1+kx.
                ky, kx = TAPS[t]
                if kx == 1:
                    return padA[:npart, ky + r_lo : ky + r_lo + rn, 2 : 2 + W]
                # read from B: A col (1+kx) == B col kx
                return padB[:npart, ky + r_lo : ky + r_lo + rn, kx : kx + W]

            # DVE depthwise for qk slabs (all taps)
            qk_blk = []
            for s in range(3):
                acc_final = blkpool.tile([128, PXB], BF16, tag=f"qkblk{s}")
                qk_blk.append(acc_final)
                seq = [dwsc[0], dwsc[1]] * 4 + [acc_final]
                prev = None
                for t in range(9):
                    w_ap = dwqk[s][:, t : t + 1]
                    cur = seq[t][:, 0:PXB]
                    w3 = win(padqkA[s][par], padqkB[s][par], 128, t, 0, R)
                    curv = cur.rearrange("p (r c) -> p r c", r=R)
                    if t == 0:
                        nc.vector.tensor_scalar(curv, w3, w_ap, None, OP.mult)
                    else:
                        nc.vector.scalar_tensor_tensor(
                            curv, w3, w_ap, prev.rearrange("p (r c) -> p r c", r=R),
                            OP.mult, OP.add,
                        )
                    prev = cur

            # PE depthwise for v slab0 (all taps), psum chunks of 4 rows
            for ci in range(2):
                ps = ps_dw.tile([128, 4, W], F32, tag="pedw")
                for ti, t in enumerate(range(9)):
                    ky, kx = TAPS[t]
                    rhs = padvA[0][par][:, ky + 4 * ci : ky + 4 * ci + 4, 1 + kx : 1 + kx + W]
                    nc.tensor.matmul(
                        ps, diagv0[:, t, :], rhs,
                        start=(ti == 0), stop=(ti == 8),
                    )
                nc.scalar.copy(
                    v_buf[0][:, (r0 + 4 * ci) * W : (r0 + 4 * ci + 4) * W].rearrange(
                        "p (r c) -> p r c", r=4
                    ),
                    ps,
                )

            # v slab1: PE partial (kx!=1), then DVE taps kx==1
            accv1 = blkpool.tile([128, PXB], BF16, tag="accv1")
            for ci in range(2):
                ps = ps_dw.tile([128, 4, W], F32, tag="pedw")
                for ti, t in enumerate(V1_PE_TAPS):
                    ky, kx = TAPS[t]
                    rhs = padvA[1][par][:, ky + 4 * ci : ky + 4 * ci + 4, 1 + kx : 1 + kx + W]
                    nc.tensor.matmul(
                        ps, diagv1[:, t, :], rhs,
                        start=(ti == 0), stop=(ti == len(V1_PE_TAPS) - 1),
                    )
                nc.scalar.copy(
                    accv1[:, 512 * ci : 512 * (ci + 1)].rearrange("p (r c) -> p r c", r=4),
                    ps,
                )
            vdst = v_buf[1][:, r0 * W : (r0 + R) * W]
            chain1 = [dwsc[0][:, 0:PXB], dwsc[1][:, 0:PXB], vdst]
            prev = accv1[:, 0:PXB]
            for ti, t in enumerate(V1_DVE_TAPS):
                cur = chain1[ti] if ti < len(V1_DVE_TAPS) - 1 else vdst
                nc.vector.scalar_tensor_tensor(
                    cur.rearrange("p (r c) -> p r c", r=R),
                    win(padvA[1][par], None, 128, t, 0, R),
                    dwv1[:, t : t + 1],
                    prev.rearrange("p (r c) -> p r c", r=R),
                    OP.mult, OP.add,
                )
                prev = cur

            # transpose q|k block -> [px, 384] groups via DMA xbar
            qkT = blkpool.tile([128, 8 * O], BF16, tag="qkT")
            for s in range(3):
                for g in range(8):
                    nc.sync.dma_start_transpose(
                        qkT[:, g * O + 128 * s : g * O + 128 * (s + 1)],
                        qk_blk[s][:, 128 * g : 128 * (g + 1)],
                    )

            # stacked gram accumulation
            for g in range(8):
                for h in range(HEADS):
                    sl = qkT[:, g * O + 96 * h : g * O + 96 * (h + 1)]
                    nc.tensor.matmul(
                        gram[:, 96 * h : 96 * (h + 1)], sl, sl,
                        start=(b == 0 and g == 0), stop=(b == NB - 1 and g == 7),
                        skip_group_check=True,
                    )

        # ---------------- phase B ----------------
        ps_dw.release()
        ps_conv.release()
        ps_misc = tc.alloc_tile_pool(name="ps_misc", bufs=1, space="PSUM")

        gram_sb = pb_pool.tile([96, 4 * 96], F32, tag="gram_sb")
        nc.vector.tensor_copy(gram_sb[:], gram[:])
        msk = pb_pool.tile([96, 4 * 96], F32, tag="msk")
        nc.vector.tensor_mul(msk[:], gram_sb[:], gmask[:])
        ss = pb_pool.tile([96, 4], F32, tag="ss")
        nc.vector.tensor_reduce(
            ss[:], msk[:].rearrange("p (h n) -> p h n", h=4), mybir.AxisListType.X, OP.add
        )
        rs = pb_pool.tile([96, 4], F32, tag="rs")
        nc.scalar.activation(rs[:], ss[:], AF.Sqrt)
        nc.vector.reciprocal(rs[:], rs[:])
        nc.vector.tensor_mul(rs[:], rs[:], tmmask[:])  # fold temperature into q rows

        # row form of rs: [4, 96]
        ps_t = ps_misc.tile([128, 128], F32, tag="ps_misc")
        nc.tensor.transpose(ps_t[:4, :96], rs[:], idf32[:96, :96])
        rs_row = pb_pool.tile([4, 96], F32, tag="rs_row")
        nc.vector.tensor_copy(rs_row[:], ps_t[:4, :96])

        # column-scale tensor via selector matmuls: cs[h][c,d] = rs_k[h][d]
        ps_cs = ps_misc.tile([48, 4 * 48], F32, tag="ps_misc")
        for h in range(HEADS):
            nc.tensor.matmul(
                ps_cs[:, 48 * h : 48 * (h + 1)], hsel[:, 48 * h : 48 * (h + 1)],
                rs_row[:, 48:96], start=True, stop=True,
            )

        # S = G_qk * rs_q*temp (rows) * rs_k (cols)
        S = pb_pool.tile([48, 4 * 48], F32, tag="S")
        for h in range(HEADS):
            nc.vector.tensor_scalar(
                S[:, 48 * h : 48 * (h + 1)],
                gram_sb[0:48, 96 * h + 48 : 96 * h + 96],
                rs[0:48, h : h + 1],
                None, OP.mult,
            )
        nc.vector.tensor_mul(S[:], S[:], ps_cs[:])
        P = pb_pool.tile([48, 4 * 48], F32, tag="P")
        nc.scalar.activation(P[:], S[:], AF.Exp)
        den = pb_pool.tile([48, 4], F32, tag="den")
        nc.vector.tensor_reduce(
            den[:], P[:].rearrange("p (h n) -> p h n", h=4), mybir.AxisListType.X, OP.add
        )
        nc.vector.reciprocal(den[:], den[:])
        A = pb_pool.tile([48, 4 * 48], BF16, tag="A")
        for h in range(HEADS):
            nc.vector.tensor_scalar(
                A[:, 48 * h : 48 * (h + 1)], P[:, 48 * h : 48 * (h + 1)],
                den[:, h : h + 1], None, OP.mult,
            )
        # AT: head h -> rows 64*(h%2):+48 of col block h//2 (matches v_buf layout)
        AT = pb_pool.tile([128, 2, 48], BF16, tag="AT")
        for h in range(HEADS):
            ps_at = ps_misc.tile([128, 48], BF16, tag="ps_at", name=f"ps_at{h}")
            rlo = 64 * (h % 2)
            nc.tensor.transpose(
                ps_at[rlo : rlo + 48, :], A[:, 48 * h : 48 * (h + 1)], idbf[:48, :48]
            )
            nc.vector.tensor_copy(AT[rlo : rlo + 48, h // 2, :], ps_at[rlo : rlo + 48, :])

        # attn @ v -> proj -> out, in 512-px chunks
        ps_misc.release()
        ps_gram.release()
        ps_o = tc.alloc_tile_pool(name="ps_o", bufs=2, space="PSUM")
        NCH = HW // 512
        # attn-out staging with dead rows zeroed once (proj weights are zero there too)
        ao0 = [pb_pool.tile([128, 512], BF16, tag=f"ao0_{p}", name=f"ao0_{p}") for p in range(2)]
        ao1 = [pb_pool.tile([128, 512], BF16, tag=f"ao1_{p}", name=f"ao1_{p}") for p in range(2)]
        for p in range(2):
            for ao in (ao0[p], ao1[p]):
                # partition starts must be 32-aligned; live rows get overwritten
                nc.vector.memset(ao[32:64, :], 0.0)
                nc.vector.memset(ao[96:128, :], 0.0)
        for ci in range(NCH):
            px = ci * 512
            par = ci % 2
            po0 = ps_o.tile([128, 512], F32, tag="po0")
            po1 = ps_o.tile([128, 512], F32, tag="po1")
            v0 = v_buf[0][:, px : px + 512]
            v1 = v_buf[1][:, px : px + 512]
            nc.tensor.matmul(po0[0:48, :], AT[0:48, 0, :], v0[0:48, :], start=True, stop=True)
            nc.tensor.matmul(po0[64:112, :], AT[64:112, 0, :], v0[64:112, :], start=True, stop=True)
            nc.tensor.matmul(po1[0:48, :], AT[0:48, 1, :], v1[0:48, :], start=True, stop=True)
            nc.tensor.matmul(po1[64:112, :], AT[64:112, 1, :], v1[64:112, :], start=True, stop=True)

            nc.vector.tensor_copy(ao0[par][0:48, :], po0[0:48, :])
            nc.vector.tensor_copy(ao0[par][64:112, :], po0[64:112, :])
            nc.vector.tensor_copy(ao1[par][0:48, :], po1[0:48, :])
            nc.vector.tensor_copy(ao1[par][64:112, :], po1[64:112, :])

            py_a = ps_o.tile([128, 512], F32, tag="py_a")
            py_b = ps_o.tile([64, 512], F32, tag="py_b")
            nc.tensor.matmul(py_a[:], wppad_a[:, 0:128], ao0[par][:], start=True, stop=False)
            nc.tensor.matmul(py_a[:], wppad_b[:, 0:128], ao1[par][:], start=False, stop=True)
            nc.tensor.matmul(py_b[:], wppad_a[:, 128:192], ao0[par][:], start=True, stop=False)
            nc.tensor.matmul(py_b[:], wppad_b[:, 128:192], ao1[par][:], start=False, stop=True)

            ys_a = pb_pool.tile([128, 512], F32, tag="ys_a")
            ys_b = pb_pool.tile([64, 512], F32, tag="ys_b")
            nc.scalar.copy(ys_a[:], py_a[:])
            nc.scalar.copy(ys_b[:], py_b[:])
            nc.sync.dma_start(d_y[0:128, px : px + 512], ys_a[:])
            nc.sync.dma_start(d_y[128:192, px : px + 512], ys_b[:])

        ps_o.release()
        pb_pool.release()
        scpool.release()
        blkpool.release()
        xpool.release()
        persist.release()
        consts.release()

    nc.compile()
    return nc


# ---------------- host side ----------------
_CACHE = {}


def _prep_static(W_qk, W_qk_dw, W_v, W_v_dw, W_proj, temperature):
    # head-interleaved channel permutation for qk: [q_h|k_h] blocks of 96
    perm = np.zeros(O, np.int64)
    for h in range(HEADS):
        perm[96 * h : 96 * h + 48] = np.arange(48 * h, 48 * h + 48)
        perm[96 * h + 48 : 96 * h + 96] = 192 + np.arange(48 * h, 48 * h + 48)

    wqkT = np.ascontiguousarray(W_qk[:, :, 0, 0].T[:, perm]).astype(np.float32)
    dwqk = np.ascontiguousarray(W_qk_dw[:, 0].reshape(O, 9)[perm]).astype(np.float32)

    # v channels padded to 64-aligned head slots: new chan (s,r): head 2s+r//64,
    # within-head idx r%64 (<48 live, else dead/zero). 256 slots = 2 slabs x 128.
    live = np.zeros(256, np.bool_)
    src_ch = np.zeros(256, np.int64)
    for s in range(2):
        for j in range(2):
            h = 2 * s + j
            r = 128 * s + 64 * j
            live[r : r + 48] = True
            src_ch[r : r + 48] = 48 * h + np.arange(48)

    wvT_orig = W_v[:, :, 0, 0].T.astype(np.float32)   # [192 in, 192 out]
    wvT = np.zeros((C, 256), np.float32)
    wvT[:, live] = wvT_orig[:, src_ch[live]]

    dwv_orig = W_v_dw[:, 0].reshape(C, 9).astype(np.float32)
    dwv = np.zeros((256, 9), np.float32)
    dwv[live] = dwv_orig[src_ch[live]]

    diagv0 = np.zeros((128, 9, 128), np.float32)
    diagv1 = np.zeros((128, 9, 128), np.float32)
    for t in range(9):
        diagv0[np.arange(128), t, np.arange(128)] = dwv[0:128, t]
        diagv1[np.arange(128), t, np.arange(128)] = dwv[128:256, t]

    # proj weights with rows in the padded attn-out channel order (zeros on dead)
    wprojT_orig = W_proj[:, :, 0, 0].T.astype(np.float32)  # [192 in, 192 out]
    wprojT = np.zeros((256, C), np.float32)
    wprojT[live] = wprojT_orig[src_ch[live]]

    gmask = np.zeros((96, 4 * 96), np.float32)
    for h in range(HEADS):
        gmask[np.arange(96), 96 * h + np.arange(96)] = 1.0

    temp = np.asarray(temperature).reshape(HEADS)
    tmmask = np.ones((96, 4), np.float32)
    tmmask[0:48, :] = temp[None, :]

    hsel = np.zeros((4, 4 * 48), np.float32)
    for h in range(HEADS):
        hsel[h, 48 * h : 48 * (h + 1)] = 1.0

    return {
        "wqkT": wqkT,
        "wvT": wvT,
        "wprojT": wprojT.astype(bf16),
        "dwqk": dwqk,
        "dwv": dwv,
        "diagv0": diagv0.reshape(128, 9 * 128).astype(bf16),
        "diagv1": diagv1.reshape(128, 9 * 128).astype(bf16),
        "gmask": gmask,
        "tmmask": tmmask,
        "idbf": np.eye(128, dtype=bf16),
        "idf32": np.eye(128, dtype=np.float32),
        "hsel": hsel,
    }


def kernel(x, W_qk, W_qk_dw, W_v, W_v_dw, W_proj, temperature):
    x = np.asarray(x, np.float32)
    b = x.shape[0]
    assert b == 8 and x.shape[1] == C

    if "nc" not in _CACHE:
        _CACHE["nc"] = build_nc()
    nc = _CACHE["nc"]

    static = _prep_static(
        np.asarray(W_qk), np.asarray(W_qk_dw), np.asarray(W_v),
        np.asarray(W_v_dw), np.asarray(W_proj), np.asarray(temperature),
    )
    in_maps = []
    for i in range(b):
        m = dict(static)
        m["x"] = np.ascontiguousarray(x[i].reshape(C, HW))
        in_maps.append(m)

    res = run_bass_kernel_spmd(nc, in_maps, core_ids=list(range(8)))
    y = np.stack([res.results[i]["y"].reshape(C, H, W) for i in range(8)])
    return y.astype(np.float32)


if __name__ == "__main__":
    t0 = time.time()
    nc = build_nc()
    print(f"build+compile: {time.time()-t0:.1f}s")
